# revision 1
# baseline (speedup 1.0000x reference)
"""Trainium2 Bass kernel for nn_AttentionRegression (ragged segment attention).

Math reformulation (exact):
  y[b] = g_x*f_x[b] + g_b + num[b]/den[b]
    w_t   = n_t . g_n                     (g weights applied per neighbour row)
    z_t   = exp(sigmoid(tanh(n_t @ W1n^T + f_x[seg]*w1x + b1) @ W2 + b2))
    num_b = sum_{t in seg b} z_t * w_t ;  den_b = sum z_t
  (softmax max-subtraction dropped: scores are sigmoid outputs in (0,1), so
   exp() is stable and the ratio is mathematically unchanged.)

Device layout: segments sorted by length into 16 strata; stratum k supplies one
128-segment block to each of the 8 cores, padded to a common length Ls[k]
(multiple of 8). Neighbours are shipped transposed+bf16 as nt[128 feat, col]
with col = blockbase + pos*128 + seg_local, so the per-row scalars computed by
the PE land as [seg=partition, pos=free] and segment sums are free-dim reduces.

Per 128-row tile the PE does LDWEIGHTS(nt tile) + matmul against a [128,13]
weight block (12 cols = W1n^T, col 12 = g_n). The per-sample bias fx*w1x + b1
is injected by a K=2 rank-1 matmul into the same PSUM accumulation group.
"""

import numpy as np
import ml_dtypes
from contextlib import ExitStack

import concourse.bass as bass
import concourse.bacc as bacc
import concourse.tile as tile
from concourse import mybir
from concourse.bass_utils import run_bass_kernel_spmd

B, T, NF, H = 16384, 1048576, 128, 12
NCORES = 8
SEGS_PER_BLOCK = 128
CH = 32  # positions per superchunk (psum [128, 13*CH])
F32 = mybir.dt.float32
BF16 = mybir.dt.bfloat16
AL = mybir.AluOpType
AF = mybir.ActivationFunctionType

_program_cache = {}


def build_program(Ls, nblk, nrep=1, dual_dma=False, ch=CH):
    nc = bacc.Bacc(
        "TRN2",
        target_bir_lowering=False,
        debug=False,
        enable_asserts=False,
    )
    sumL = sum(Ls)
    R = 128 * sumL
    nt = nc.dram_tensor("nt", [128, R], BF16, kind="ExternalInput").ap()
    w13 = nc.dram_tensor("w13", [128, 13], BF16, kind="ExternalInput").ap()
    w2rep = nc.dram_tensor("w2rep", [128, CH * H], BF16, kind="ExternalInput").ap()
    # aux3 cols: 0 b2/2 | 1 gx | 2 gb
    aux3 = nc.dram_tensor("aux3", [128, 3], F32, kind="ExternalInput").ap()
    fxd = nc.dram_tensor("fx", [128, nblk], F32, kind="ExternalInput").ap()
    fxT2 = nc.dram_tensor("fxT2", [2, 128 * nblk], BF16, kind="ExternalInput").ap()
    bpat = nc.dram_tensor("bpat", [2, 13 * CH], BF16, kind="ExternalInput").ap()
    maskd = nc.dram_tensor("mask", [128, sumL], BF16, kind="ExternalInput").ap()
    yd = nc.dram_tensor("y", [128, nblk], F32, kind="ExternalOutput").ap()

    with tile.TileContext(nc) as tc, ExitStack() as ctx:
        if nrep > 1:
            ctx.enter_context(tc.For_i(0, nrep, 1, name="bench"))
        singles = ctx.enter_context(tc.tile_pool(name="singles", bufs=1))
        bigp = ctx.enter_context(tc.tile_pool(name="bigp", bufs=4))
        psp = ctx.enter_context(tc.tile_pool(name="psp", bufs=4, space="PSUM"))
        hp = ctx.enter_context(tc.tile_pool(name="hp", bufs=3))

        # small loads ride the gpsimd SWDGE queue so the SP queue can start
        # streaming neighbour blocks immediately
        w13_s = singles.tile([128, 13], BF16)
        nc.gpsimd.dma_start(out=w13_s[:], in_=w13)
        w2rep_s = singles.tile([128, CH * H], BF16)
        nc.gpsimd.dma_start(out=w2rep_s[:], in_=w2rep)
        aux3_s = singles.tile([128, 3], F32)
        nc.gpsimd.dma_start(out=aux3_s[:], in_=aux3)
        fx_s = singles.tile([128, nblk], F32)
        nc.gpsimd.dma_start(out=fx_s[:], in_=fxd)
        fxT2_s = singles.tile([2, 128 * nblk], BF16)
        nc.gpsimd.dma_start(out=fxT2_s[:], in_=fxT2)
        bpat_s = singles.tile([2, 13 * CH], BF16)
        nc.gpsimd.dma_start(out=bpat_s[:], in_=bpat)
        mask_s = singles.tile([128, sumL], BF16)
        nc.gpsimd.dma_start(out=mask_s[:], in_=maskd)

        s_all = singles.tile([128, sumL], F32)
        w_all = singles.tile([128, sumL], F32)
        den_all = singles.tile([128, nblk], F32)
        num_all = singles.tile([128, nblk], F32)

        col = 0
        soff = 0
        nchunk = 0
        for g in range(nblk):
            L = Ls[g]
            for p0 in range(0, L, ch):
                c = min(ch, L - p0)
                ntb = bigp.tile([128, 128 * c], BF16, tag="ntb")
                eng = nc.gpsimd if (dual_dma and nchunk % 2) else nc.sync
                nchunk += 1
                eng.dma_start(
                    out=ntb[:],
                    in_=nt[:, col + p0 * 128: col + (p0 + c) * 128])
                ps = psp.tile([128, 13 * c], F32, tag="ps")
                # rank-1 bias opens the bank group:
                #   psum[p, 13*i+j] = fx[p]*w1x[j] + b1[j]  (0 at j=12)
                nc.tensor.matmul(
                    ps[:, 0: 13 * c],
                    lhsT=fxT2_s[:, g * 128:(g + 1) * 128],
                    rhs=bpat_s[:, 0: 13 * c],
                    start=True, stop=False, skip_group_check=True)
                for i in range(c):
                    nc.tensor.matmul(
                        ps[:, 13 * i: 13 * (i + 1)],
                        lhsT=ntb[:, i * 128: (i + 1) * 128],
                        rhs=w13_s[:], start=False, stop=(i == c - 1),
                        skip_group_check=True)
                psv = ps[:].rearrange("p (c t) -> p c t", t=13)
                th = hp.tile([128, c * H], BF16, tag="th")
                nc.scalar.activation(
                    out=th[:].rearrange("p (c t) -> p c t", t=H),
                    in_=psv[:, :, 0:12], func=AF.Tanh)
                m = hp.tile([128, c * H], BF16, tag="m")
                nc.vector.tensor_mul(m[:], th[:], w2rep_s[:, 0: c * H])
                nc.vector.reduce_sum(
                    out=s_all[:, soff + p0: soff + p0 + c],
                    in_=m[:].rearrange("p (c t) -> p c t", t=H),
                    axis=mybir.AxisListType.X)
                nc.scalar.activation(
                    out=w_all[:, soff + p0: soff + p0 + c],
                    in_=psv[:, :, 12], func=AF.Copy)

            # per-block epilogue, fully inside the {Tanh, Exp, Copy} func set:
            # sigmoid(x) = 0.5 + 0.5*tanh(x/2) and softmax drops constants, so
            # z = exp(0.5*tanh(0.5*(s + b2))) has the exact softmax ratios.
            u = hp.tile([128, L], F32, tag="u")
            nc.scalar.activation(out=u[:], in_=s_all[:, soff: soff + L],
                                 func=AF.Tanh, bias=aux3_s[:, 0:1], scale=0.5)
            z = hp.tile([128, L], F32, tag="z")
            nc.scalar.activation(out=z[:], in_=u[:], func=AF.Exp, scale=0.5)
            zm = hp.tile([128, L], F32, tag="zm")
            nc.vector.tensor_mul(zm[:], z[:], mask_s[:, soff: soff + L])
            zw = hp.tile([128, L], F32, tag="zw")
            nc.vector.tensor_mul(zw[:], zm[:], w_all[:, soff: soff + L])
            nc.vector.reduce_sum(out=den_all[:, g:g + 1], in_=zm[:],
                                 axis=mybir.AxisListType.X)
            nc.vector.reduce_sum(out=num_all[:, g:g + 1], in_=zw[:],
                                 axis=mybir.AxisListType.X)
            col += 128 * L
            soff += L

        den_eps = singles.tile([128, nblk], F32)
        nc.vector.tensor_scalar(
            out=den_eps[:], in0=den_all[:], scalar1=1e-30, scalar2=None,
            op0=AL.add)
        rec_all = singles.tile([128, nblk], F32)
        nc.vector.reciprocal(out=rec_all[:], in_=den_eps[:])
        t_all = singles.tile([128, nblk], F32)
        nc.vector.tensor_mul(t_all[:], num_all[:], rec_all[:])
        y1_all = singles.tile([128, nblk], F32)
        nc.vector.scalar_tensor_tensor(
            out=y1_all[:], in0=fx_s[:], scalar=aux3_s[:, 1:2], in1=t_all[:],
            op0=AL.mult, op1=AL.add)
        y_all = singles.tile([128, nblk], F32)
        nc.vector.tensor_scalar(
            out=y_all[:], in0=y1_all[:], scalar1=aux3_s[:, 2:3], scalar2=None,
            op0=AL.add)
        nc.sync.dma_start(out=yd, in_=y_all[:])
    nc.compile()
    return nc, R


def prep_host(f_x, neighbours, seg_ids, f_W1, f_b1, f_W2, f_b2, g_W, g_b):
    """Shard/pack inputs. Returns (Ls, nblk, in_maps, order)."""
    lens_all = np.bincount(seg_ids, minlength=B).astype(np.int64)
    order = np.argsort(-lens_all, kind="stable")
    nblk = B // (SEGS_PER_BLOCK * NCORES)  # 16
    stratum = SEGS_PER_BLOCK * NCORES  # 1024
    Ls = []
    for k in range(nblk):
        m = int(lens_all[order[k * stratum:(k + 1) * stratum]].max())
        Ls.append(max(1, m))
    sumL = sum(Ls)
    R = 128 * sumL

    row_start = np.zeros(B + 1, np.int64)
    row_start[1:] = np.cumsum(lens_all)
    nbf = neighbours.astype(ml_dtypes.bfloat16)

    w13 = np.zeros((128, 13), np.float32)
    w13[:, 0:12] = f_W1[:, 1:].T
    w13[:, 12] = g_W[0, 1:]
    w13 = w13.astype(ml_dtypes.bfloat16)

    w1x = f_W1[:, 0].astype(np.float32)
    w2rep = np.tile(np.concatenate([f_W2[0], ]).astype(np.float32), CH)
    w2rep = np.tile(w2rep[None, :], (128, 1)).astype(ml_dtypes.bfloat16)

    aux3 = np.zeros((128, 3), np.float32)
    aux3[:, 0] = 0.5 * f_b2[0]
    aux3[:, 1] = g_W[0, 0]
    aux3[:, 2] = g_b[0]

    bpat = np.zeros((2, 13 * CH), np.float32)
    bpat[0, :] = np.tile(np.concatenate([w1x, [0.0]]).astype(np.float32), CH)
    bpat[1, :] = np.tile(np.concatenate([f_b1, [0.0]]).astype(np.float32), CH)
    bpat = bpat.astype(ml_dtypes.bfloat16)

    in_maps = []
    for c in range(NCORES):
        idx = np.empty(R, np.int64)
        valid = np.empty(R, bool)
        fx_mat = np.empty((128, nblk), np.float32)
        mask = np.empty((128, sumL), ml_dtypes.bfloat16)
        off = 0
        soff = 0
        for g in range(nblk):
            Lg = Ls[g]
            gids = order[g * stratum + 128 * c: g * stratum + 128 * (c + 1)]
            pos = np.arange(Lg)[:, None]
            rows = row_start[gids][None, :] + pos          # [Lg, 128]
            val = pos < lens_all[gids][None, :]
            blockn = Lg * 128
            idx[off:off + blockn] = np.where(val, rows, 0).reshape(-1)
            valid[off:off + blockn] = val.reshape(-1)
            fx_mat[:, g] = f_x[gids, 0]
            mask[:, soff:soff + Lg] = val.T.astype(ml_dtypes.bfloat16)
            off += blockn
            soff += Lg
        nrows = nbf[idx]                                   # [R, 128] bf16
        nrows[~valid] = ml_dtypes.bfloat16(0)
        nt_c = np.ascontiguousarray(nrows.T)               # [128, R]
        fxT2 = np.ones((2, 128 * nblk), np.float32)
        fxT2[0, :] = fx_mat.T.reshape(-1)                  # block-major
        fxT2 = fxT2.astype(ml_dtypes.bfloat16)
        in_maps.append({
            "nt": nt_c, "w13": w13, "w2rep": w2rep, "aux3": aux3,
            "fx": fx_mat, "fxT2": fxT2, "bpat": bpat, "mask": mask,
        })
    return Ls, nblk, in_maps, order


def assemble_output(results, order, nblk):
    stratum = SEGS_PER_BLOCK * NCORES
    y_full = np.empty(B, np.float32)
    for c in range(NCORES):
        yc = results[c]["y"]  # [128, nblk]
        for g in range(nblk):
            y_full[order[g * stratum + 128 * c: g * stratum + 128 * (c + 1)]] = yc[:, g]
    return y_full[:, None]


def kernel(**inputs) -> np.ndarray:
    args = {k: np.asarray(v) for k, v in inputs.items()}
    Ls, nblk, in_maps, order = prep_host(
        args["f_x"], args["neighbours"], args["seg_ids"],
        args["f_W1"], args["f_b1"], args["f_W2"], args["f_b2"],
        args["g_W"], args["g_b"])
    key = (tuple(Ls), nblk)
    if key not in _program_cache:
        _program_cache[key] = build_program(Ls, nblk)
    nc, _ = _program_cache[key]
    res = run_bass_kernel_spmd(nc, in_maps, core_ids=list(range(NCORES)))
    return assemble_output(res.results, order, nblk)

